# revision 1
# baseline (speedup 1.0000x reference)
"""Additive-attention (ContentAttender) Bass kernel for 8 TRN2 NeuronCores.

Problem: B=4, NQ=512, NK=512, D=128, H=32
  kh = keys @ Wk; qh = queries @ Wq
  logits[b,q,k] = w2 . tanh(qh[b,q] + kh[b,k] + b1) + b2
  out = softmax_k(logits) @ keys

Sharding: data-parallel over (batch x query-half) -> 8 cores, each core
handles one batch's 256 queries vs all 512 keys. No collectives.

Per-core pipeline: queries packed 4-per-32-partition-group; the (q,k,h)
tanh tensor is built as 64 DVE broadcast-adds (khT4 + per-group query
bias, bf16 2x mode) feeding big-chunk ScalarE tanh (the roofline:
~4.2M tanh elems/core at 1 elem/cycle/lane), and the h-contraction with
w2 runs on the TensorEngine via a host-built block-diagonal weight
matrix into 32-row PSUM slices. Softmax skips max-subtraction
(|logits| <= sum|w2| ~ 3, safe in fp32); b2 dropped (softmax
shift-invariant). Normalization deferred: context = (exp @ keys) / rowsum.
Bank A's softmax exp runs in the mid-stream ACT gap; its transposes and
context matmuls are deferred past the final logits matmuls so the
terminal exp's TensorEngine dependencies are never queued behind them.
"""

import contextlib

import numpy as np
import ml_dtypes

import concourse.bass as bass  # noqa: F401
import concourse.mybir as mybir
import concourse.tile as tile
from concourse import bacc
from concourse.bass_utils import run_bass_kernel_spmd

F32 = mybir.dt.float32
BF16 = mybir.dt.bfloat16
AF = mybir.ActivationFunctionType

B, NQ, NK, D, H = 4, 512, 512, 128, 32
NQC = NQ // 2          # queries per core = 256
NG = NQC // 4          # query groups per core = 64

# bundleA columns: keysT | queriesT | Wk | Wq | b14
KT0, QT0, WK0, WQ0, B140 = 0, 512, 768, 800, 832
NCOLA = 833
# bundleB columns: kctx | W2D | identity
KX0, W2D0, ID0 = 0, 512, 768
NCOLB = 896

CHUNKS = [4, 8, 12, 16, 8, 8, 4, 3, 1]  # groups per tanh chunk (sum = 64)

_CACHED_NC = None


def _build_nc():
    nc = bacc.Bacc("TRN2", target_bir_lowering=False, debug=False)

    bundleA = nc.declare_dram_parameter("bundleA", [128, NCOLA], BF16, isOutput=False)
    bundleB = nc.declare_dram_parameter("bundleB", [128, NCOLB], BF16, isOutput=False)
    out = nc.declare_dram_parameter("out", [NQC, D], F32, isOutput=True)

    with tile.TileContext(nc) as tc, contextlib.ExitStack() as ctx:
        cpool = ctx.enter_context(tc.tile_pool(name="consts", bufs=1))
        spool = ctx.enter_context(tc.tile_pool(name="schunk", bufs=3))
        epool = ctx.enter_context(tc.tile_pool(name="softmax", bufs=2))
        ps_kh = ctx.enter_context(tc.tile_pool(name="ps_kh", bufs=1, space="PSUM"))
        ps_qb = ctx.enter_context(tc.tile_pool(name="ps_qb", bufs=1, space="PSUM"))
        ps_logits = ctx.enter_context(
            tc.tile_pool(name="ps_logits", bufs=2, space="PSUM")
        )
        ps_tr = ctx.enter_context(tc.tile_pool(name="ps_tr", bufs=2, space="PSUM"))
        ps_ctx = ctx.enter_context(tc.tile_pool(name="ps_ctx", bufs=2, space="PSUM"))

        bA = cpool.tile([128, NCOLA], BF16, tag="bA")
        nc.sync.dma_start(bA[:], bundleA[:])
        bB = cpool.tile([128, NCOLB], BF16, tag="bB")
        nc.sync.dma_start(bB[:], bundleB[:])

        kT = bA[:, KT0 : KT0 + NK]
        qT = bA[:, QT0 : QT0 + NQC]
        Wk_sb = bA[:, WK0 : WK0 + H]
        Wq_sb = bA[:, WQ0 : WQ0 + H]
        b14 = bA[:, B140 : B140 + 1]
        kctx_sb = bB[:, KX0 : KX0 + NK]
        W2D_sb = bB[:, W2D0 : W2D0 + 8 * H]
        id_sb = bB[:, ID0 : ID0 + 128]

        # khT4[(j,h), k] = (keys @ Wk)[k, h] replicated on 4 partition groups
        khT4_ps = ps_kh.tile([128, NK], F32, tag="khps", name="khT4_ps")
        for j in range(4):
            nc.tensor.matmul(
                khT4_ps[32 * j : 32 * j + 32, :],
                Wk_sb,
                kT,
                start=True,
                stop=True,
                tile_position=(0, 32 * j),
            )
        khT4 = cpool.tile([128, NK], BF16, tag="khT4")
        nc.vector.tensor_copy(khT4[:], khT4_ps[:])

        # QB4[(j,h), g] = qh[64j + g, h] + b1[h]   (b1 folded on copy-out)
        b14f = cpool.tile([128, 1], F32, tag="b14f")
        nc.vector.tensor_copy(b14f[:], b14)
        QB4_ps = ps_qb.tile([128, NG], F32, tag="qbps", name="QB4_ps")
        for j in range(4):
            nc.tensor.matmul(
                QB4_ps[32 * j : 32 * j + 32, :],
                Wq_sb,
                qT[:, NG * j : NG * (j + 1)],
                start=True,
                stop=True,
                tile_position=(0, 32 * j),
            )
        QB4 = cpool.tile([128, NG], F32, tag="QB4")
        nc.vector.tensor_scalar_add(QB4[:], QB4_ps[:], b14f[:])

        logits_ps = [None, None]
        g0 = 0

        def emit_chunk(n, bias_mode=False):
            nonlocal g0
            T = spool.tile([128, max(CHUNKS) * NK], BF16, tag="T", name="T")
            if bias_mode:
                # tanh straight off khT4 with the query bias in the ACT
                # affine stage: no DVE adds on the pipeline-ramp chunk.
                for gl in range(n):
                    g = g0 + gl
                    nc.scalar.activation(
                        T[:, NK * gl : NK * (gl + 1)],
                        khT4[:],
                        AF.Tanh,
                        bias=QB4[:, g : g + 1],
                    )
            else:
                S = spool.tile([128, max(CHUNKS) * NK], BF16, tag="S", name="S")
                for gl in range(n):
                    g = g0 + gl
                    nc.vector.tensor_scalar_add(
                        S[:, NK * gl : NK * (gl + 1)], khT4[:], QB4[:, g : g + 1]
                    )
                nc.scalar.activation(T[:, : NK * n], S[:, : NK * n], AF.Tanh)
            for gl in range(n):
                g = g0 + gl
                beta = g // 32
                s = (g // 8) % 4
                g8 = g % 8
                if logits_ps[beta] is None:
                    logits_ps[beta] = ps_logits.tile(
                        [128, NK], F32, tag="logits", name=f"logits{beta}"
                    )
                nc.tensor.matmul(
                    logits_ps[beta][32 * s : 32 * s + 32, :],
                    W2D_sb[:, 32 * g8 : 32 * g8 + 32],
                    T[:, NK * gl : NK * (gl + 1)],
                    start=(g8 == 0),
                    stop=(g8 == 7),
                    tile_position=(0, 32 * s),
                )
            g0 += n

        tails = {}

        def emit_tail_exp(beta):
            E = epool.tile([128, NK], BF16, tag="E", name="E")
            rs = epool.tile([128, 1], F32, tag="rs", name="rs")
            nc.scalar.activation(E[:], logits_ps[beta][:], AF.Exp, accum_out=rs[:])
            rr = epool.tile([128, 1], F32, tag="rr", name="rr")
            nc.vector.reciprocal(rr[:], rs[:])
            tails[beta] = (E, rr)

        def emit_tail_rest(beta):
            E, rr = tails[beta]
            ET = epool.tile([128, NK], BF16, tag="ET", name="ET")
            for t in range(4):
                trp = ps_tr.tile([128, 128], BF16, tag="tr", name="trp")
                nc.tensor.transpose(trp[:], E[:, 128 * t : 128 * (t + 1)], id_sb)
                nc.vector.tensor_copy(ET[:, 128 * t : 128 * (t + 1)], trp[:])
            ctxp = ps_ctx.tile([128, D], F32, tag="ctx", name="ctxp")
            for t in range(4):
                nc.tensor.matmul(
                    ctxp[:],
                    ET[:, 128 * t : 128 * (t + 1)],
                    kctx_sb[:, 128 * t : 128 * (t + 1)],
                    start=(t == 0),
                    stop=(t == 3),
                )
            ctx_sb = epool.tile([128, D], F32, tag="ctxs", name="ctx_sb")
            nc.vector.tensor_scalar_mul(ctx_sb[:], ctxp[:], rr[:])
            nc.sync.dma_start(out[128 * beta : 128 * (beta + 1), :], ctx_sb[:])

        # Bank A's exp fits the ACT gap after chunk 4, but its PE work
        # (transposes + context matmuls) is deferred until after the final
        # chunks so the terminal logits matmuls (expB's dependency) are not
        # queued behind it on the TensorEngine.
        for ci, n in enumerate(CHUNKS):
            emit_chunk(n, bias_mode=(ci == 0))
            if ci == 4:
                emit_tail_exp(0)
        emit_tail_exp(1)
        emit_tail_rest(0)
        emit_tail_rest(1)

    nc.compile()
    return nc


def _get_nc():
    global _CACHED_NC
    if _CACHED_NC is None:
        _CACHED_NC = _build_nc()
    return _CACHED_NC


def _build_w2d(w2):
    """(128, 256): slice g8 has column 4*g8+j = w2 on partitions [32j, 32j+32)."""
    w2d = np.zeros((128, 8 * H), np.float32)
    for g8 in range(8):
        for j in range(4):
            w2d[32 * j : 32 * j + 32, 32 * g8 + 4 * g8 + j] = w2
    return w2d


def _qmap():
    """out row r -> local query index."""
    r = np.arange(NQC)
    beta = r // 128
    p = r % 128
    return 64 * (p % 4) + 32 * beta + 8 * (p // 32) + (p % 32) // 4


def _in_maps(keys, queries, Wk, Wq, b1, w2):
    keys = np.asarray(keys, np.float32)
    queries = np.asarray(queries, np.float32)
    Wk = np.asarray(Wk, np.float32)
    Wq = np.asarray(Wq, np.float32)
    b1 = np.asarray(b1, np.float32)
    w2 = np.asarray(w2, np.float32)

    bundleB = np.zeros((128, NCOLB), np.float32)
    bundleB[:, W2D0 : W2D0 + 8 * H] = _build_w2d(w2)
    bundleB[:, ID0 : ID0 + 128] = np.eye(128, dtype=np.float32)
    b14 = np.tile(b1, 4)  # (128,)

    maps = []
    for c in range(8):
        b, half = divmod(c, 2)
        kb = keys[b]  # (512, 128)
        bA = np.zeros((128, NCOLA), np.float32)
        bA[:, KT0 : KT0 + NK] = kb.T
        bA[:, QT0 : QT0 + NQC] = queries[b, NQC * half : NQC * (half + 1)].T
        bA[:, WK0 : WK0 + H] = Wk
        bA[:, WQ0 : WQ0 + H] = Wq
        bA[:, B140] = b14
        bB = bundleB.copy()
        bB[:, KX0 : KX0 + NK] = (
            kb.reshape(4, 128, 128).transpose(1, 0, 2).reshape(128, 512)
        )
        maps.append(
            {
                "bundleA": bA.astype(ml_dtypes.bfloat16),
                "bundleB": bB.astype(ml_dtypes.bfloat16),
            }
        )
    return maps


def _run(in_maps, trace=False):
    nc = _get_nc()
    return run_bass_kernel_spmd(nc, in_maps, core_ids=list(range(8)), trace=trace)


def kernel(keys, queries, Wk, Wq, b1, w2, b2):
    res = _run(_in_maps(keys, queries, Wk, Wq, b1, w2))
    qmap = _qmap()
    outv = np.empty((B, NQ, D), np.float32)
    for c in range(8):
        b, half = divmod(c, 2)
        outv[b, NQC * half + qmap] = res.results[c]["out"]
    return outv



# revision 5
# speedup vs baseline: 2.2436x; 2.2436x over previous
"""Additive-attention (ContentAttender) Bass kernel for 8 TRN2 NeuronCores.

Problem: B=4, NQ=512, NK=512, D=128, H=32
  kh = keys @ Wk; qh = queries @ Wq
  logits[b,q,k] = w2 . tanh(qh[b,q] + kh[b,k] + b1) + b2
  out = softmax_k(logits) @ keys

Sharding: data-parallel over (batch x query-half) -> 8 cores, each core
handles one batch's 256 queries vs all 512 keys. No collectives.

Algorithm: the non-separable tanh(a+c) (4.2M ACT elems/core in the naive
form -- the old roofline) is replaced by a separable sine expansion

  tanh(x) ~= sum_j beta_j sin(j*om*x),   j = 1..4
  sin(j*om*(a+c)) = S_j(a)C_j(c) + C_j(a)S_j(c)

so logits become ONE TensorEngine contraction of dim 32h x 8 = 256.
Only S1,C1,S2,C2 are evaluated directly by the ACT Sin table (args kept
within its [-pi,pi] valid range); S3,C3,S4,C4 come from one stride-2
Chebyshev step F2 = 2*cos(2*om*u) . F1 - F0 on the Vector engine in bf16
(2x/4x DVE modes). b1 is folded into the ACT bias vectors (c-side),
w2*beta_j into the a-side bf16 copies. beta is fit at runtime by
least-squares against the bf16-realized basis on sampled (a,c) pairs, so
quantization bias is absorbed. Softmax skips max-subtraction
(|logits| <= sum_j|beta_j|*sum|w2| ~ 1.3); b2 dropped (shift-invariant).
Normalization deferred: context = (exp @ keys) / rowsum.
"""

import contextlib

import numpy as np
import ml_dtypes

import concourse.bass as bass  # noqa: F401
import concourse.mybir as mybir
import concourse.tile as tile
from concourse import bacc
from concourse.bass_utils import run_bass_kernel_spmd

F32 = mybir.dt.float32
BF16 = mybir.dt.bfloat16
AF = mybir.ActivationFunctionType

B, NQ, NK, D, H = 4, 512, 512, 128, 32
NQC = NQ // 2          # queries per core = 256
NJ = 4                 # sine harmonics

# bundleA columns: keysT | queriesT | Wk | Wq
KT0, QT0, WK0, WQ0 = 0, 512, 768, 800
NCOLA = 832
# bundleB columns: kctx | identity
KX0, ID0 = 0, 512
NCOLB = 640
# consts columns (fp32)
C_SA1, C_BA1, C_SC1, C_BC1, C_BMC, C_WB1, C_WB2, C_PM1A, C_PM1C, C_S2, C_PI2 = range(11)
NCC = 11

_CACHED_NC = None


def _build_nc():
    nc = bacc.Bacc("TRN2", target_bir_lowering=False, debug=False)

    bundleA = nc.declare_dram_parameter("bundleA", [128, NCOLA], BF16, isOutput=False)
    bundleB = nc.declare_dram_parameter("bundleB", [128, NCOLB], BF16, isOutput=False)
    consts = nc.declare_dram_parameter("consts", [128, NCC], F32, isOutput=False)
    out = nc.declare_dram_parameter("out", [NQC, D], F32, isOutput=True)

    with tile.TileContext(nc) as tc, contextlib.ExitStack() as ctx:
        cpool = ctx.enter_context(tc.tile_pool(name="consts", bufs=1))
        epool = ctx.enter_context(tc.tile_pool(name="softmax", bufs=2))
        ps_kh = ctx.enter_context(tc.tile_pool(name="ps_kh", bufs=1, space="PSUM"))
        ps_qh = ctx.enter_context(tc.tile_pool(name="ps_qh", bufs=1, space="PSUM"))
        ps_logits = ctx.enter_context(
            tc.tile_pool(name="ps_logits", bufs=2, space="PSUM")
        )
        ps_tr = ctx.enter_context(tc.tile_pool(name="ps_tr", bufs=2, space="PSUM"))
        ps_ctx = ctx.enter_context(tc.tile_pool(name="ps_ctx", bufs=2, space="PSUM"))

        cc = cpool.tile([128, NCC], F32, tag="cc")
        nc.sync.dma_start(cc[:], consts[:])
        bA = cpool.tile([128, NCOLA], BF16, tag="bA")
        nc.sync.dma_start(bA[:], bundleA[:])
        bB = cpool.tile([128, NCOLB], BF16, tag="bB")
        nc.sync.dma_start(bB[:], bundleB[:])

        kT = bA[:, KT0 : KT0 + NK]
        qT = bA[:, QT0 : QT0 + NQC]
        Wk_sb = bA[:, WK0 : WK0 + H]
        Wq_sb = bA[:, WQ0 : WQ0 + H]
        kctx_sb = bB[:, KX0 : KX0 + NK]
        id_sb = bB[:, ID0 : ID0 + 128]

        # Replicated kh^T / qh^T: rows (j*32+h) = (keys@Wk)[k,h] etc., 4 copies
        kh_ps = ps_kh.tile([128, NK], F32, tag="khps", name="kh_ps")
        for j in range(4):
            nc.tensor.matmul(
                kh_ps[32 * j : 32 * j + 32, :],
                Wk_sb,
                kT,
                start=True,
                stop=True,
                tile_position=(0, 32 * j),
            )
        qh_ps = ps_qh.tile([128, NQC], F32, tag="qhps", name="qh_ps")
        for j in range(4):
            nc.tensor.matmul(
                qh_ps[32 * j : 32 * j + 32, :],
                Wq_sb,
                qT,
                start=True,
                stop=True,
                tile_position=(0, 32 * j),
            )

        # --- ACT: base sine bands (bf16 out) ---
        # Mpc = cos(2om(c+b1)) replicated; Fc1 = [C1; S1; C2; S2] (c-side)
        Mpc = cpool.tile([128, NK], BF16, tag="Mpc")
        nc.scalar.activation(
            Mpc[:], kh_ps[:], AF.Sin, bias=cc[:, C_BMC : C_BMC + 1], scale=cc[:, C_S2 : C_S2 + 1]
        )
        Fc1 = cpool.tile([128, NK], BF16, tag="Fc1")
        nc.scalar.activation(
            Fc1[0:64, :],
            kh_ps[0:64, :],
            AF.Sin,
            bias=cc[0:64, C_BC1 : C_BC1 + 1],
            scale=cc[0:64, C_SC1 : C_SC1 + 1],
        )
        nc.scalar.activation(
            Fc1[96:128, :],
            kh_ps[96:128, :],
            AF.Sin,
            bias=cc[96:128, C_BC1 : C_BC1 + 1],
            scale=cc[96:128, C_SC1 : C_SC1 + 1],
        )
        # Mpa = cos(2om*a) replicated; F1a = [S1; C1; S2; C2] (a-side)
        Mpa = cpool.tile([128, NQC], BF16, tag="Mpa")
        nc.scalar.activation(
            Mpa[:], qh_ps[:], AF.Sin, bias=cc[:, C_PI2 : C_PI2 + 1], scale=cc[:, C_S2 : C_S2 + 1]
        )
        F1a = cpool.tile([128, NQC], BF16, tag="F1a")
        nc.scalar.activation(
            F1a[0:96, :],
            qh_ps[0:96, :],
            AF.Sin,
            bias=cc[0:96, C_BA1 : C_BA1 + 1],
            scale=cc[0:96, C_SA1 : C_SA1 + 1],
        )

        # band fills: F1a band3 = C2 (from Mpa), Fc1 band2 = C2 (from Mpc)
        nc.gpsimd.tensor_copy(F1a[96:128, :], Mpa[96:128, :])
        nc.gpsimd.tensor_copy(Fc1[64:96, :], Mpc[64:96, :])

        # F0 tiles: stride-2 predecessors
        F0a = cpool.tile([128, NQC], BF16, tag="F0a")  # [-S1; C1; 0; 1]
        nc.vector.tensor_scalar_mul(F0a[0:64, :], F1a[0:64, :], cc[0:64, C_PM1A : C_PM1A + 1])
        nc.gpsimd.memset(F0a[64:96, :], 0.0)
        nc.gpsimd.memset(F0a[96:128, :], 1.0)
        F0c = cpool.tile([128, NK], BF16, tag="F0c")   # [C1; -S1; 1; 0]
        nc.vector.tensor_scalar_mul(F0c[0:64, :], Fc1[0:64, :], cc[0:64, C_PM1C : C_PM1C + 1])
        nc.gpsimd.memset(F0c[64:96, :], 1.0)
        nc.gpsimd.memset(F0c[96:128, :], 0.0)

        # M = 2*Mp
        Ma = cpool.tile([128, NQC], BF16, tag="Ma")
        nc.vector.tensor_scalar_mul(Ma[:], Mpa[:], 2.0)
        Mc = cpool.tile([128, NK], BF16, tag="Mc")
        nc.vector.tensor_scalar_mul(Mc[:], Mpc[:], 2.0)

        # one stride-2 Chebyshev step: F2 = M . F1 - F0  (bands S3,C3,S4,C4)
        tmpa = cpool.tile([128, NQC], BF16, tag="tmpa")
        nc.vector.tensor_mul(tmpa[:], Ma[:], F1a[:])
        F2a = cpool.tile([128, NQC], BF16, tag="F2a")
        nc.vector.tensor_sub(out=F2a[:], in0=tmpa[:], in1=F0a[:])
        tmpc = cpool.tile([128, NK], BF16, tag="tmpc")
        nc.vector.tensor_mul(tmpc[:], Mc[:], Fc1[:])
        F2c = cpool.tile([128, NK], BF16, tag="F2c")
        nc.vector.tensor_sub(out=F2c[:], in0=tmpc[:], in1=F0c[:])

        # a-side scaled copies: fold w2*beta_j
        Ua1 = cpool.tile([128, NQC], BF16, tag="Ua1")
        nc.vector.tensor_scalar_mul(Ua1[:], F1a[:], cc[:, C_WB1 : C_WB1 + 1])
        Ua2 = cpool.tile([128, NQC], BF16, tag="Ua2")
        nc.vector.tensor_scalar_mul(Ua2[:], F2a[:], cc[:, C_WB2 : C_WB2 + 1])

        # logits[q,k] per 128-query block: 2 chained matmuls (contraction 256)
        logits_ps = []
        for blk in range(2):
            lp = ps_logits.tile([128, NK], F32, tag="logits", name=f"logits{blk}")
            nc.tensor.matmul(
                lp[:], Ua1[:, 128 * blk : 128 * blk + 128], Fc1[:],
                start=True, stop=False,
            )
            nc.tensor.matmul(
                lp[:], Ua2[:, 128 * blk : 128 * blk + 128], F2c[:],
                start=False, stop=True,
            )
            logits_ps.append(lp)

        tails = {}

        def emit_tail_exp(blk):
            E = epool.tile([128, NK], BF16, tag="E", name="E")
            rs = epool.tile([128, 1], F32, tag="rs", name="rs")
            nc.scalar.activation(E[:], logits_ps[blk][:], AF.Exp, accum_out=rs[:])
            rr = epool.tile([128, 1], F32, tag="rr", name="rr")
            nc.vector.reciprocal(rr[:], rs[:])
            tails[blk] = (E, rr)

        def emit_tail_rest(blk):
            E, rr = tails[blk]
            ET = epool.tile([128, NK], BF16, tag="ET", name="ET")
            for t in range(4):
                trp = ps_tr.tile([128, 128], BF16, tag="tr", name="trp")
                nc.tensor.transpose(trp[:], E[:, 128 * t : 128 * (t + 1)], id_sb)
                nc.vector.tensor_copy(ET[:, 128 * t : 128 * (t + 1)], trp[:])
            ctxp = ps_ctx.tile([128, D], F32, tag="ctx", name="ctxp")
            for t in range(4):
                nc.tensor.matmul(
                    ctxp[:],
                    ET[:, 128 * t : 128 * (t + 1)],
                    kctx_sb[:, 128 * t : 128 * (t + 1)],
                    start=(t == 0),
                    stop=(t == 3),
                )
            ctx_sb = epool.tile([128, D], F32, tag="ctxs", name="ctx_sb")
            nc.vector.tensor_scalar_mul(ctx_sb[:], ctxp[:], rr[:])
            nc.sync.dma_start(out[128 * blk : 128 * (blk + 1), :], ctx_sb[:])

        emit_tail_exp(0)
        emit_tail_exp(1)
        emit_tail_rest(0)
        emit_tail_rest(1)

    nc.compile()
    return nc


def _get_nc():
    global _CACHED_NC
    if _CACHED_NC is None:
        _CACHED_NC = _build_nc()
    return _CACHED_NC


def _bf(x):
    return np.asarray(x, ml_dtypes.bfloat16).astype(np.float32)


def _fit(qh, kh, b1, om):
    """Least-squares beta against the bf16-realized sine basis."""
    def chains(u):
        S1 = _bf(np.sin(om * u)); C1 = _bf(np.sin(om * u + np.pi / 2))
        Mp = _bf(np.sin(2 * om * u + np.pi / 2))
        S2 = _bf(np.sin(2 * om * u)); C2 = Mp
        M = _bf(2.0 * Mp)
        S3 = _bf(_bf(M * S1) - (-S1)); C3 = _bf(_bf(M * C1) - C1)
        S4 = _bf(_bf(M * S2) - 0.0);   C4 = _bf(_bf(M * C2) - 1.0)
        return (S1, C1), (S2, C2), (S3, C3), (S4, C4)

    a_ch = chains(qh.reshape(-1, H))
    c_ch = chains(kh.reshape(-1, H) + b1)
    rng = np.random.default_rng(12345)
    n_s = 120000
    ii = rng.integers(0, qh.reshape(-1, H).shape[0], n_s)
    kk = rng.integers(0, kh.reshape(-1, H).shape[0], n_s)
    hh = rng.integers(0, H, n_s)
    x = qh.reshape(-1, H)[ii, hh] + kh.reshape(-1, H)[kk, hh] + b1[hh]
    Phi = np.empty((n_s, NJ), np.float64)
    for j in range(NJ):
        Sa, Ca = a_ch[j]
        Sc, Cc = c_ch[j]
        Phi[:, j] = Sa[ii, hh] * Cc[kk, hh] + Ca[ii, hh] * Sc[kk, hh]
    beta = np.linalg.lstsq(Phi, np.tanh(x), rcond=None)[0]
    return beta.astype(np.float32)


def _in_maps(keys, queries, Wk, Wq, b1, w2):
    keys = np.asarray(keys, np.float32)
    queries = np.asarray(queries, np.float32)
    Wk = np.asarray(Wk, np.float32)
    Wq = np.asarray(Wq, np.float32)
    b1 = np.asarray(b1, np.float32)
    w2 = np.asarray(w2, np.float32)

    # host model of the device-side qh/kh (bf16 operands, fp32 accum)
    qh = _bf(queries) @ _bf(Wq)
    kh = _bf(keys) @ _bf(Wk)
    Amax = float(np.abs(qh).max())
    Cmax = float(np.abs(kh + b1).max())
    SAFE = 3.05
    om = min((SAFE - np.pi / 2) / max(Amax, Cmax), SAFE / (2 * max(Amax, Cmax)))
    beta = _fit(qh, kh, b1, om)

    # consts (fp32)
    ccv = np.zeros((128, NCC), np.float32)
    band = np.repeat([0, 1, 2, 3], 32)
    b14 = np.tile(b1, 4)
    ccv[:, C_SA1] = np.where(band < 2, om, 2 * om)
    ccv[:, C_BA1] = np.where((band % 2) == 1, np.pi / 2, 0.0)   # [S1;C1;S2;C2]
    ccv[:, C_SC1] = np.where(band < 2, om, 2 * om)
    # c-side bands [C1; S1; C2; S2]: bias = j*om*b1 + (pi/2 on cos bands)
    jmul = np.where(band < 2, om, 2 * om)
    cosb = (band == 0) | (band == 2)
    ccv[:, C_BC1] = jmul * b14 + np.where(cosb, np.pi / 2, 0.0)
    ccv[:, C_BMC] = 2 * om * b14 + np.pi / 2
    wb = np.empty(128, np.float32)
    wb[0:32] = w2 * beta[0]; wb[32:64] = w2 * beta[0]
    wb[64:96] = w2 * beta[1]; wb[96:128] = w2 * beta[1]
    ccv[:, C_WB1] = wb
    wb2v = np.empty(128, np.float32)
    wb2v[0:32] = w2 * beta[2]; wb2v[32:64] = w2 * beta[2]
    wb2v[64:96] = w2 * beta[3]; wb2v[96:128] = w2 * beta[3]
    ccv[:, C_WB2] = wb2v
    ccv[0:32, C_PM1A] = -1.0; ccv[32:64, C_PM1A] = 1.0
    ccv[:, C_S2] = 2 * om
    ccv[:, C_PI2] = np.pi / 2
    ccv[0:32, C_PM1C] = 1.0; ccv[32:64, C_PM1C] = -1.0

    bundleB = np.zeros((128, NCOLB), np.float32)
    bundleB[:, ID0 : ID0 + 128] = np.eye(128, dtype=np.float32)

    maps = []
    for c in range(8):
        b, half = divmod(c, 2)
        kb = keys[b]  # (512, 128)
        bA = np.zeros((128, NCOLA), np.float32)
        bA[:, KT0 : KT0 + NK] = kb.T
        bA[:, QT0 : QT0 + NQC] = queries[b, NQC * half : NQC * (half + 1)].T
        bA[:, WK0 : WK0 + H] = Wk
        bA[:, WQ0 : WQ0 + H] = Wq
        bB = bundleB.copy()
        bB[:, KX0 : KX0 + NK] = (
            kb.reshape(4, 128, 128).transpose(1, 0, 2).reshape(128, 512)
        )
        maps.append(
            {
                "bundleA": bA.astype(ml_dtypes.bfloat16),
                "bundleB": bB.astype(ml_dtypes.bfloat16),
                "consts": ccv,
            }
        )
    return maps


def _run(in_maps, trace=False):
    nc = _get_nc()
    return run_bass_kernel_spmd(nc, in_maps, core_ids=list(range(8)), trace=trace)


def kernel(keys, queries, Wk, Wq, b1, w2, b2):
    res = _run(_in_maps(keys, queries, Wk, Wq, b1, w2))
    outv = np.empty((B, NQ, D), np.float32)
    for c in range(8):
        b, half = divmod(c, 2)
        outv[b, NQC * half : NQC * (half + 1)] = res.results[c]["out"]
    return outv
